# revision 36
# baseline (speedup 1.0000x reference)
"""Trainium2 Bass kernel for nn_KernelFilter_S (dynamic per-sample filter CNN).

Data-parallel over batch B=8 across 8 NeuronCores (one sample per core).

Per-core math (sample x = content[b], s = style[b]):
  c1 = conv3x3(x, ds_w) + ds_b                       [32,64,64]
  pooled_F = mean_HW(conv3x3(s, cwF)) + cbF          [32]    (F = 1,2)
  filtF = (pooled_F @ fwF.T + fbF).reshape(32,32,3,3)
  c2 = leaky(conv3x3_dyn(c1, filt1), 0.2)
  c3 = conv3x3_dyn(c2, filt2)
  out = x + conv3x3(c3, up_w) + up_b                 [512,64,64]

Style trick: mean-pool-of-conv needs only 9 rectangle sums R[i,t] per
channel, not the full conv (removes the two 512->32 style convs).

Engine strategy (vs one matmul per conv tap):
  * ds conv (K=128, M=32): 4x PE column tiling -- 4 output row tiles run
    concurrently in 4 column groups of one PSUM bank (interleaved
    accumulation groups verified on HW), quadrupling PE utilization.
  * dyn convs (K=32->96): 3x3 taps grouped by kx; each intermediate
    image lives as 3 row-shifted partition bands (96 partitions) so a
    conv tile needs 3 matmuls at K=96. PSUM evacuations write band 1
    directly (cross-partition, HW-verified); bands 0/2 are made by 2
    shifted SBUF->SBUF DMAs per tile group, pipelined with the matmuls.
  * up conv: K=96 via the same bands, M=128: 3 matmuls per (tile,chunk);
    output staged bf16 and upcast to f32 on the host.
  * The filter-predictor FC uses host-permuted weight columns so the
    [p,j] -> [(g i),(kx o)] re-layout round trip through DRAM is two
    coarse-grained contiguous DMAs (no scatter packets).
  * Style stats: 9 raw sums per channel (total/edge rows/edge cols/
    corners, edges host-appended contiguously); the per-tap rectangle
    combinations are folded into the predictor weights by linearity.
  * Content DMA lands directly in the zero-padded conv layout (host
    pre-pads rows); content/style ship as bf16; inputs split across the
    sync and scalar HW DMA queues; evacuations are balanced over
    scalar/vector/gpsimd.
"""

import os
import sys
import numpy as np

sys.path.insert(0, "/opt/trn_rl_repo")

import concourse.bass as bass
import concourse.bacc as bacc
import concourse.mybir as mybir
import concourse.tile as tile
from concourse.bass_utils import run_bass_kernel_spmd

F32 = mybir.dt.float32
BF16 = mybir.dt.bfloat16
NP_BF16 = np.dtype(mybir.dt.np(BF16))

H = W = 64
PW = W + 2            # padded row width = 66
NPIX = H * W          # 4096
NROW = H * PW         # 4224 (content rows region)
GUARD = PW + 1        # 67
BUFW = GUARD + (H + 2) * PW + GUARD  # 4490
IMG0 = GUARD + PW     # buffer col where image row 0 starts (133)
CIN = 512
INNER = 32
NC_CHUNKS = CIN // 128  # 4

TAPS = [(ky, kx) for ky in range(3) for kx in range(3)]
SHIFT = [(ky - 1) * PW + (kx - 1) for ky, kx in TAPS]

ROW_TILES = [(r0, 7) for r0 in range(0, 63, 7)] + [(63, 1)]
PSN = 7 * PW  # 462
TILE_GROUPS = [[0, 1, 2, 3], [4, 5, 6, 7], [8, 9]]


def _col0(r0):
    return GUARD + (r0 + 1) * PW


def _interior(ap, nr):
    return ap.rearrange("p (r x) -> p r x", x=PW)[:, :, 1:1 + W]


def _build_program():
    nc = bacc.Bacc(None, target_bir_lowering=False)

    cont_h = nc.dram_tensor("content", [CIN, NROW], BF16, kind="ExternalInput")
    # style_aug: [.., 0:4096] image, [.., 4096:4160] col-0 gathered,
    # [.., 4160:4224] col-63 gathered, [.., 4224:4228] corners
    style_h = nc.dram_tensor("style", [CIN, NPIX + 132], BF16, kind="ExternalInput")
    # host-prelaid layouts: [p, (t c o)] so the DMA is fully contiguous
    w_ds_h = nc.dram_tensor("w_ds", [128, 9 * NC_CHUNKS * INNER], BF16,
                            kind="ExternalInput")
    w_up2_h = nc.dram_tensor("w_up2", [96, 12 * 128], BF16, kind="ExternalInput")
    cwT_h = [nc.dram_tensor(f"cwT{F}", [128, 9 * NC_CHUNKS * INNER], BF16,
                            kind="ExternalInput") for F in (1, 2)]
    fw12_h = nc.dram_tensor("fw12", [64, 288 * INNER], BF16, kind="ExternalInput")
    fb12_h = nc.dram_tensor("fb12", [128, 144], F32, kind="ExternalInput")
    biases_h = nc.dram_tensor("biases", [128, 8], F32, kind="ExternalInput")
    out_h = nc.dram_tensor("out", [CIN, NPIX], BF16, kind="ExternalOutput")
    fdram_h = [nc.dram_tensor(f"fscratch{F}", [288 * INNER], BF16, kind="Internal")
               for F in (1, 2)]

    with tile.TileContext(nc) as tc:
        with (
            tc.tile_pool(name="const", bufs=1) as const,
            tc.tile_pool(name="big", bufs=1) as big,
            tc.tile_pool(name="work", bufs=2) as work,
            tc.tile_pool(name="outp", bufs=2) as outp,
            tc.tile_pool(name="ds_ps", bufs=2, space=bass.MemorySpace.PSUM) as ds_ps,
            tc.tile_pool(name="conv_ps", bufs=4, space=bass.MemorySpace.PSUM) as conv_ps,
            tc.tile_pool(name="pred_ps", bufs=1, space=bass.MemorySpace.PSUM) as pred_ps,
        ):
            # w_ds + biases first on the scalar queue (tiny; the first
            # ds matmul needs them)
            w_ds_sb = const.tile([128, 9 * NC_CHUNKS * INNER], BF16, tag="wds")
            nc.scalar.dma_start(out=w_ds_sb[:], in_=w_ds_h[:])
            biases_sb = const.tile([128, 8], F32, tag="biases")
            nc.scalar.dma_start(out=biases_sb[:], in_=biases_h[:])

            # ---- content into padded conv layout -----------------------
            cpad = []
            for c in range(NC_CHUNKS):
                cp = big.tile([128, BUFW], BF16, tag=f"cpad{c}", name=f"cpad{c}")
                nc.gpsimd.memset(cp[:, 0:IMG0], 0.0)
                nc.gpsimd.memset(cp[:, IMG0 + NROW:BUFW], 0.0)
                eng = nc.scalar if c == 1 else nc.sync
                eng.dma_start(
                    out=cp[:, IMG0:IMG0 + NROW],
                    in_=cont_h[:].rearrange("(c p) q -> c p q", p=128)[c],
                )
                cpad.append(cp)

            # ---- weights (all contiguous, host-prelaid) ----------------
            # scalar-queue order: small urgent weights, then style chunks
            # (parallel with content on the sync queue), then big weights
            # needed only later.
            st_tiles = []
            for c in range(NC_CHUNKS):
                st = big.tile([128, NPIX + 132], BF16, tag=f"styl{c}",
                              name=f"styl{c}")
                nc.scalar.dma_start(
                    out=st[:], in_=style_h[:].rearrange("(c p) q -> c p q", p=128)[c]
                )
                st_tiles.append(st)
            cwT_sb = []
            for Fi in range(2):
                t1 = const.tile([128, 9 * NC_CHUNKS * INNER], BF16,
                                tag=f"cwT{Fi}", name=f"cwT{Fi}")
                nc.scalar.dma_start(out=t1[:], in_=cwT_h[Fi][:])
                cwT_sb.append(t1)
            fw12_sb = const.tile([64, 288 * INNER], BF16, tag="fw12")
            nc.scalar.dma_start(out=fw12_sb[:], in_=fw12_h[:])
            fb12_sb = const.tile([128, 144], F32, tag="fb12")
            nc.scalar.dma_start(out=fb12_sb[:], in_=fb12_h[:])
            w_up2_sb = const.tile([96, 12 * 128], BF16, tag="wup2")
            nc.scalar.dma_start(out=w_up2_sb[:], in_=w_up2_h[:])


            # ---- band machinery ----------------------------------------
            # band g (partitions 32g..32g+31): B_g[q] = I[q + (g-1)*PW].
            # PSUM evacuations write band 1 directly (cross-partition);
            # bands 0 and 2 are shifted copies made by 2 DMAs per group.
            def make_bands(name):
                bt = big.tile([96, BUFW], BF16, tag=name, name=name)
                nc.gpsimd.memset(bt[:], 0.0)
                return bt

            def spread_bands(bt, gi):
                grp = TILE_GROUPS[gi]
                lo = ROW_TILES[grp[0]][0]
                hi = ROW_TILES[grp[-1]][0] + ROW_TILES[grp[-1]][1]
                src = bt[32:64, IMG0 + lo * PW:IMG0 + hi * PW]
                for g in (0, 2):
                    base = IMG0 - (g - 1) * PW
                    nc.sync.dma_start(
                        out=bt[32 * g:32 * g + 32,
                               base + lo * PW:base + hi * PW],
                        in_=src,
                    )

            # ---- ds conv: content -> c1 bands (4x col-tiled) -----------
            c1b = make_bands("c1b")

            def ds_group(gi, grp):
                ps = ds_ps.tile([128, PSN], F32, tag="dsps", name=f"dsps{gi}")
                nmm = 9 * NC_CHUNKS
                k = 0
                for c in range(NC_CHUNKS):
                    for t in range(9):
                        w_ap = w_ds_sb[:, (t * NC_CHUNKS + c) * INNER:
                                       (t * NC_CHUNKS + c + 1) * INNER]
                        for j, r in enumerate(grp):
                            r0, nr = ROW_TILES[r]
                            N = nr * PW
                            nc.tensor.matmul(
                                ps[32 * j:32 * j + 32, 0:N],
                                w_ap,
                                cpad[c][:, _col0(r0) + SHIFT[t]:
                                        _col0(r0) + SHIFT[t] + N],
                                start=(k == 0), stop=(k == nmm - 1),
                                tile_position=(0, 32 * j),
                            )
                        k += 1
                for j, r in enumerate(grp):
                    r0, nr = ROW_TILES[r]
                    dst = _interior(c1b[32:64, _col0(r0):_col0(r0) + nr * PW], nr)
                    src = _interior(ps[32 * j:32 * j + 32, 0:nr * PW], nr)
                    bias = biases_sb[32 * j:32 * j + 32, 0:1]
                    if j % 2 == 0:
                        nc.scalar.activation(
                            dst, src, mybir.ActivationFunctionType.Identity,
                            bias=bias)
                    else:
                        nc.vector.tensor_scalar_add(dst, src, bias)
                spread_bands(c1b, gi)

            # ---- style stats -------------------------------------------
            # 9 raw stats per channel: total, row0, row63, col0, col63,
            # 4 corners. The tap rectangle-sum combinations are folded
            # into the host-precombined predictor weights (linearity).
            Rcb = []
            for c in range(NC_CHUNKS):
                st = st_tiles[c]
                rb = big.tile([128, 9], BF16, tag=f"rcb{c}", name=f"rcb{c}")
                half = work.tile([128, NPIX // 2], BF16, tag="sthalf")
                with nc.allow_low_precision(
                        reason="stats feed pooled/4096; bf16 ample here"):
                    # halving add runs at 2x (bf16 tensor_tensor); the 1x
                    # reduce then reads half the volume
                    nc.vector.tensor_add(half[:], st[:, 0:NPIX // 2],
                                         st[:, NPIX // 2:NPIX])
                    nc.vector.tensor_reduce(
                        rb[:, 0:1], half[:],
                        mybir.AxisListType.X, mybir.AluOpType.add)
                    for s, (lo, hi) in enumerate([(0, W),
                                                  ((H - 1) * W, NPIX),
                                                  (NPIX, NPIX + 64),
                                                  (NPIX + 64, NPIX + 128)], start=1):
                        nc.vector.tensor_reduce(
                            rb[:, s:s + 1], st[:, lo:hi],
                            mybir.AxisListType.X, mybir.AluOpType.add)
                nc.vector.tensor_copy(rb[:, 5:9], st[:, NPIX + 128:NPIX + 132])
                Rcb.append(rb)

            ds_group(0, TILE_GROUPS[0])
            ds_group(1, TILE_GROUPS[1])

            # ---- filter predictors (slotted between ds groups so the
            # DRAM round trip overlaps ds group 2 on the PE) -------------
            # pooled_F: F=0 -> (partitions 0-31, col 0); F=1 -> (32-63,
            # col 1) via col group 1; sequential accumulation groups in
            # one bank keep every evac partition-aligned.
            pooled12 = work.tile([64, 2], BF16, tag="pooled12")
            nc.vector.memset(pooled12[:], 0.0)
            pp = pred_ps.tile([64, 2], F32, tag="pp")
            for Fi in range(2):
                nmm = 9 * NC_CHUNKS
                k = 0
                for t in range(9):
                    for c in range(NC_CHUNKS):
                        j = t * NC_CHUNKS + c
                        nc.tensor.matmul(
                            pp[32 * Fi:32 * Fi + 32, Fi:Fi + 1],
                            cwT_sb[Fi][:, j * INNER:(j + 1) * INNER],
                            Rcb[c][:, t:t + 1],
                            start=(k == 0), stop=(k == nmm - 1),
                            tile_position=(0, 32 * Fi),
                        )
                        k += 1
                nc.scalar.activation(
                    pooled12[32 * Fi:32 * Fi + 32, Fi:Fi + 1],
                    pp[32 * Fi:32 * Fi + 32, Fi:Fi + 1],
                    mybir.ActivationFunctionType.Identity,
                    bias=biases_sb[32 * Fi:32 * Fi + 32, 5:6],
                    scale=1.0 / NPIX,
                )
            # fps: both filters at once; matmul j fills cols [2j, 2j+2)
            fps = pred_ps.tile([128, 144], F32, tag="fps")
            for jj in range(72):
                nc.tensor.matmul(
                    fps[:, 2 * jj:2 * jj + 2],
                    fw12_sb[:, jj * 128:(jj + 1) * 128],
                    pooled12[:],
                    start=True, stop=True,
                )
            # Filter round trip through DRAM re-lays [p, j] -> [(g i), (kx o)].
            # fw12 columns are host-permuted so BOTH hops are coarse-grained:
            # column n=j*128+p holds the filter element (r, q) = divmod(72p+j,
            # 96); the DRAM scratch is then written p-major (contiguous from
            # fsbF) and read r-major (contiguous into ft).
            ft = []
            for Fi in range(2):
                fsbF = work.tile([128, 72], BF16, tag=f"fsb{Fi}", name=f"fsbF{Fi}")
                nc.vector.tensor_add(
                    fsbF[:].rearrange("p (j u) -> p j u", u=1),
                    fps[:].rearrange("p (j f) -> p j f", f=2)[:, :, Fi:Fi + 1],
                    fb12_sb[:, 72 * Fi:72 * (Fi + 1)].rearrange("p (j u) -> p j u", u=1),
                )
                nc.scalar.dma_start(
                    out=fdram_h[Fi][:].rearrange("(p j) -> p j", p=128),
                    in_=fsbF[:],
                )
                f_t = const.tile([96, 96], BF16, tag=f"filt{Fi}", name=f"filt{Fi}")
                nc.scalar.dma_start(
                    out=f_t[:],
                    in_=fdram_h[Fi][:].rearrange("(r q) -> r q", r=96),
                )
                ft.append(f_t)

            ds_group(2, TILE_GROUPS[2])

            # ---- dyn convs (K=96, col-tiled groups) --------------------
            def dyn_conv(src_b, filt, dst_bt, leaky, label):
                for gi, grp in enumerate(TILE_GROUPS):
                    ps = conv_ps.tile([128, PSN], F32, tag="cps",
                                      name=f"dyn{label}_{gi}")
                    for kx in range(3):
                        for j, r in enumerate(grp):
                            r0, nr = ROW_TILES[r]
                            N = nr * PW
                            nc.tensor.matmul(
                                ps[32 * j:32 * j + 32, 0:N],
                                filt[:, kx * 32:(kx + 1) * 32],
                                src_b[0:96, _col0(r0) + kx - 1:
                                      _col0(r0) + kx - 1 + N],
                                start=(kx == 0), stop=(kx == 2),
                                tile_position=(0, 32 * j),
                            )
                    npart = 32 * len(grp)
                    if leaky:
                        nc.scalar.activation(
                            ps[0:npart, :], ps[0:npart, :],
                            mybir.ActivationFunctionType.Lrelu, alpha=0.2,
                            bias=biases_sb[0:npart, 6:7])
                    for j, r in enumerate(grp):
                        r0, nr = ROW_TILES[r]
                        dst = _interior(
                            dst_bt[32:64, _col0(r0):_col0(r0) + nr * PW], nr)
                        src = _interior(ps[32 * j:32 * j + 32, 0:nr * PW], nr)
                        if j % 2 == 0:
                            nc.vector.tensor_copy(dst, src)
                        else:
                            nc.scalar.activation(
                                dst, src, mybir.ActivationFunctionType.Copy)
                    spread_bands(dst_bt, gi)

            c2b = make_bands("c2b")
            dyn_conv(c1b, ft[0], c2b, True, 'a')
            c3b = make_bands("c3b")
            dyn_conv(c2b, ft[1], c3b, False, 'b')

            # ---- up conv (K=96) + residual add -> out ------------------
            # bf16 output staging, one big half-chunk DMA per 5 row tiles
            # (host upcasts to f32); evacuations split vector/scalar+gpsimd
            for cc in range(NC_CHUNKS):
                stage = outp.tile([128, NPIX], BF16, tag="ostage",
                                  name=f"ost{cc}")
                out_cc = out_h[:].rearrange("(c p) q -> c p q", p=128)[cc]
                for ri, (r0, nr) in enumerate(ROW_TILES):
                    N = nr * PW
                    ps = conv_ps.tile([128, PSN], F32, tag="cps",
                                      name=f"up_{cc}_{r0}")
                    for kx in range(3):
                        nc.tensor.matmul(
                            ps[:, 0:N],
                            w_up2_sb[:, (kx * 4 + cc) * 128:(kx * 4 + cc + 1) * 128],
                            c3b[0:96, _col0(r0) + kx - 1:_col0(r0) + kx - 1 + N],
                            start=(kx == 0), stop=(kx == 2),
                        )
                    sview = stage[:, r0 * W:(r0 + nr) * W] \
                        .rearrange("p (r x) -> p r x", x=W)
                    with nc.allow_low_precision(
                            reason="bf16 output staging; host upcasts"):
                        if ri % 3 != 2:
                            nc.vector.scalar_tensor_tensor(
                                sview,
                                _interior(ps[:, 0:N], nr),
                                biases_sb[:, 1 + cc:2 + cc],
                                _interior(cpad[cc][:, _col0(r0):_col0(r0) + N], nr),
                                op0=mybir.AluOpType.add,
                                op1=mybir.AluOpType.add,
                            )
                        else:
                            nc.scalar.activation(
                                sview,
                                _interior(ps[:, 0:N], nr),
                                mybir.ActivationFunctionType.Identity,
                                bias=biases_sb[:, 1 + cc:2 + cc],
                            )
                            nc.gpsimd.tensor_add(
                                sview, sview,
                                _interior(cpad[cc][:, _col0(r0):_col0(r0) + N], nr),
                            )
                    if ri == 4:
                        nc.sync.dma_start(out=out_cc[:, 0:35 * W],
                                          in_=stage[:, 0:35 * W])
                    elif ri == 6:
                        nc.sync.dma_start(out=out_cc[:, 35 * W:49 * W],
                                          in_=stage[:, 35 * W:49 * W])
                nc.sync.dma_start(out=out_cc[:, 49 * W:NPIX],
                                  in_=stage[:, 49 * W:NPIX])

    nc.compile()
    return nc


_NC_CACHE = None


def _get_nc():
    global _NC_CACHE
    if _NC_CACHE is None:
        _NC_CACHE = _build_program()
    return _NC_CACHE


# fps matmul j computes fw12[:, j*128+p].T @ pooled -> value stored at
# DRAM offset d = 72p + j, read back as ft[(g i), (kx o)] = fdram[96r + q].
# So column n = j*128+p of fw12 must hold the fw row for filter element
# (r, q) = divmod(72p + j, 96).
_N = np.arange(72 * 128)
_J, _P = _N // 128, _N % 128
_R, _Q = np.divmod(72 * _P + _J, 96)
_G, _I = _R // 32, _R % 32
_KX, _O = _Q // 32, _Q % 32
_MORIG = _O * 288 + _I * 9 + (_G * 3 + _KX)


def _stats_weights(cw):
    # Fold the 9 tap rectangle-sum combinations into 9 stat-weight
    # matrices W_s[o, i] (linearity of the pooled-conv):
    #   R[t] = total - rowsub - colsub + corner  per tap.
    W = np.stack([
        cw.sum((2, 3)),                  # total
        -cw[:, :, 2, :].sum(-1),         # uses row-0 subtraction (ky==2)
        -cw[:, :, 0, :].sum(-1),         # row 63 (ky==0)
        -cw[:, :, :, 2].sum(-1),         # col 0 (kx==2)
        -cw[:, :, :, 0].sum(-1),         # col 63 (kx==0)
        cw[:, :, 2, 2],                  # corner (0,0)
        cw[:, :, 2, 0],                  # corner (0,63)
        cw[:, :, 0, 2],                  # corner (63,0)
        cw[:, :, 0, 0],                  # corner (63,63)
    ])                                   # [9, o, i]
    # layout [p, (s*4+c)*32+o] = W[s, o, c*128+p]
    return np.ascontiguousarray(
        W.reshape(9, INNER, NC_CHUNKS, 128)
         .transpose(3, 0, 2, 1).reshape(128, 9 * NC_CHUNKS * INNER)
    ).astype(NP_BF16)


def _prep_weights(ds_w, up_w, f1_cw, f1_fw, f2_cw, f2_fw):
    # w_ds[p, (t*4+c)*32+o] = ds_w[o, c*128+p, t]
    w_ds = np.ascontiguousarray(
        ds_w.reshape(INNER, NC_CHUNKS, 128, 9)
            .transpose(2, 3, 1, 0).reshape(128, 9 * NC_CHUNKS * INNER)
    ).astype(NP_BF16)
    cwT = [_stats_weights(cw) for cw in (f1_cw, f2_cw)]
    # w_up2[g*32+i, (kx*4+cc)*128+oc] = up_w[cc*128+oc, i, g, kx]
    w_up2 = np.ascontiguousarray(
        up_w.reshape(NC_CHUNKS, 128, INNER, 3, 3)
            .transpose(3, 2, 4, 0, 1)            # [g, i, kx, cc, oc]
            .reshape(96, 12 * 128)).astype(NP_BF16)
    fw12 = np.ascontiguousarray(np.concatenate(
        [fw[_MORIG, :].T for fw in (f1_fw, f2_fw)], axis=0)).astype(NP_BF16)
    return w_ds, w_up2, cwT, fw12


def _prep_fb12(f1_fb, f2_fb):
    # fb12[p, 72*F + j] = fb_F[m_orig(n = j*128+p)]
    out = np.zeros((128, 144), np.float32)
    for Fi, fb in enumerate((f1_fb, f2_fb)):
        out[:, 72 * Fi:72 * (Fi + 1)] = fb[_MORIG].reshape(72, 128).T
    return np.ascontiguousarray(out)


def _pad_content(img):
    # [512, 64, 64] -> [512, 64*66] with zero col pads
    p = np.zeros((CIN, H, PW), np.float32)
    p[:, :, 1:1 + W] = img
    return p.reshape(CIN, NROW).astype(NP_BF16)


def kernel(content, style, ds_w, ds_b, up_w, up_b,
           f1_cw, f1_cb, f1_fw, f1_fb,
           f2_cw, f2_cb, f2_fw, f2_fb):
    content = np.asarray(content, np.float32)
    style = np.asarray(style, np.float32)
    B = content.shape[0]
    assert B == 8

    w_ds, w_up2, cwT, fw12 = _prep_weights(
        np.asarray(ds_w, np.float32), np.asarray(up_w, np.float32),
        np.asarray(f1_cw, np.float32), np.asarray(f1_fw, np.float32),
        np.asarray(f2_cw, np.float32), np.asarray(f2_fw, np.float32))
    fb12 = _prep_fb12(np.asarray(f1_fb, np.float32), np.asarray(f2_fb, np.float32))
    biases = np.zeros((128, 8), np.float32)
    biases[:, 0] = np.tile(np.asarray(ds_b, np.float32), 4)
    biases[:, 1:5] = np.asarray(up_b, np.float32).reshape(NC_CHUNKS, 128).T
    biases[0:64, 5] = np.concatenate([np.asarray(f1_cb, np.float32),
                                      np.asarray(f2_cb, np.float32)])

    shared = {
        "w_ds": w_ds, "w_up2": w_up2,
        "cwT1": cwT[0], "cwT2": cwT[1],
        "fw12": fw12, "fb12": fb12, "biases": biases,
    }
    in_maps = []
    for b in range(B):
        m = dict(shared)
        m["content"] = _pad_content(content[b].reshape(CIN, H, W))
        sb_ = style[b].reshape(CIN, H, W)
        aug = np.concatenate([
            sb_.reshape(CIN, NPIX),
            sb_[:, :, 0], sb_[:, :, W - 1],
            sb_[:, 0, 0:1], sb_[:, 0, W - 1:W],
            sb_[:, H - 1, 0:1], sb_[:, H - 1, W - 1:W],
        ], axis=1)
        m["style"] = np.ascontiguousarray(aug).astype(NP_BF16)
        in_maps.append(m)

    nc = _get_nc()
    trace = bool(int(os.environ.get("KF_TRACE", "0")))
    res = run_bass_kernel_spmd(nc, in_maps, core_ids=list(range(B)), trace=trace)
    if trace and getattr(res, "exec_time_ns", None) is not None:
        print(f"HW exec time: {res.exec_time_ns} ns")
        kernel.last_exec_ns = res.exec_time_ns
    kernel.last_results = res
    out = np.stack([res.results[b]["out"].reshape(CIN, H, W) for b in range(B)])
    return out.astype(np.float32)


if __name__ == "__main__":
    _get_nc()
    print("program built + compiled OK")
